# revision 34
# baseline (speedup 1.0000x reference)
"""Trainium2 Bass kernel for multi-head attention (B=2, P=2048, M=1024, N=16, H=64).

Sharding: 8 cores = 2 batches x 4 head-groups. Core c handles batch c//4,
heads [4*(c%4), 4*(c%4)+4). Each core computes its heads' attention and the
partial output projection; the host sums partials across the 4 cores of each
batch.

Device algorithm (per core; matmul dtype selectable bf16/fp32r):
  - q^T,k^T,v^T [h', p] via projections with x^T as the moving operand,
    head-pairs concatenated to fill 128 partitions; bias added via K=1 matmul.
    One weight load feeds 4 accumulating p-tiles (LDWEIGHTS amortized).
  - scores^T [pk, pq] per head; strictly-lower-triangular keep mask (pq < pk)
    exploited by skipping fully-masked tiles and narrowing partial ones.
    Two pk-chunks of scores land in one [128,1024] PSUM tile so a single
    ScalarE exp instruction covers both (amortizes ACT fixed overhead).
  - v transposed head-wise on the PE with an appended ones row, so the z
    matmul (z_aug^T = v_aug^T @ exp^T) also yields the softmax denominators.
  - z_aug^T is PE-transposed to [pq, h] layout where the denominator is a
    per-partition scalar: reciprocal + tensor_scalar normalize, then
    PE-transposed back and head-pairs packed to K=128 for the output
    projection, which accumulates both pairs in PSUM. This per-unit work is
    interleaved with the attention stream to keep the PE fed while ScalarE
    runs exp.
  - The fully-masked query row P-1 (softmax of all -1e10 = uniform) is
    patched analytically on the host.
"""
import os
import sys

import numpy as np

if "/opt/trn_rl_repo" not in sys.path:
    sys.path.insert(0, "/opt/trn_rl_repo")

import concourse.bacc as bacc
import concourse.tile as tile
from concourse import mybir
from concourse import bass_utils
import ml_dtypes

B, P, M, N, H = 2, 2048, 1024, 16, 64
NCORES = 8
HPC = 4          # heads per core
NPAIRS = 2       # head pairs per core
MK = M // 128    # 8 contraction chunks for projections
PT = P // 512    # 4 free-dim tiles of 512 over sequence
PC = P // 128    # 16 partition chunks over sequence
MT = M // 512    # 2 output m-tiles

F32 = mybir.dt.float32
F32R = mybir.dt.float32r
BF16 = mybir.dt.bfloat16
EXP = mybir.ActivationFunctionType.Exp
MULT = mybir.AluOpType.mult

DT_MODE = os.environ.get("KERNEL_DT", "bf16")   # "bf16" | "f32r"
DT_MM = BF16 if DT_MODE == "bf16" else F32R
NP_MM = ml_dtypes.bfloat16 if DT_MODE == "bf16" else np.float32

_BUILT = {}


def _emit(nc, tc, aps, ctx):
    xT = aps["xT"]          # [1024, 2048]
    outp = aps["outp"]      # [2048, 1024]

    consts = ctx.enter_context(tc.tile_pool(name="consts", bufs=1))
    xpool = ctx.enter_context(tc.tile_pool(name="xpool", bufs=MK))
    qkpool = ctx.enter_context(tc.tile_pool(name="qkpool", bufs=2))
    vapool = ctx.enter_context(tc.tile_pool(name="vapool", bufs=68))
    zppool = ctx.enter_context(tc.tile_pool(name="zppool", bufs=16))
    expool = ctx.enter_context(
        tc.tile_pool(name="expool", bufs=(9 if DT_MODE == "bf16" else 9)))
    zsbpool = ctx.enter_context(tc.tile_pool(name="zsbpool", bufs=6))
    znpool = ctx.enter_context(tc.tile_pool(name="znpool", bufs=6))
    rcpool = ctx.enter_context(tc.tile_pool(name="rcpool", bufs=8))
    opool = ctx.enter_context(tc.tile_pool(name="opool", bufs=4))

    eye = consts.tile([128, 128], F32)
    nc.sync.dma_start(eye[:], aps["eye"][:])
    mask = consts.tile([128, 128], DT_MM)
    nc.sync.dma_start(mask[:], aps["mask"][:])
    ones32 = consts.tile([1, 512], F32)
    nc.vector.memset(ones32[:], 1.0)
    if DT_MODE == "bf16":
        ones_mm = consts.tile([1, 512], BF16)
        nc.vector.memset(ones_mm[:], 1.0)
    else:
        ones_mm = consts.tile([1, 512], F32R)
        nc.vector.tensor_copy(ones_mm[:], ones32[:])
    wos = []
    for pr in range(NPAIRS):
        wot = consts.tile([128, 1024], DT_MM, tag=f"wo{pr}", name=f"wo{pr}")
        nc.sync.dma_start(wot[:], aps["wo"][pr])
        wos.append(wot)

    # x^T chunks [128 m, 2048 p]
    xsb = []
    for k in range(MK):
        xt = xpool.tile([128, 2048], DT_MM, tag="x")
        eng = nc.sync if k % 2 == 0 else nc.scalar
        eng.dma_start(xt[:], xT[128 * k:128 * (k + 1), :])
        xsb.append(xt)

    tiles = {}
    qts, kts = {}, {}

    def finish_pair(pr, j, zpss, t_pool):
        """Copy both heads' z_aug^T out of PSUM, then normalize in pq-space
        with the two heads' transpose chains interleaved (hides the
        PE->DVE->PE latency of each chain)."""
        zsbs = []
        for h01 in range(2):
            zsb = zsbpool.tile([65, 512], F32, tag="z",
                               name=f"zsb{pr}_{h01}_{j}")
            nc.vector.tensor_copy(zsb[:], zpss[h01][:])
            if j == PT - 1:
                # fully-masked query row P-1: denom 0 -> 1 so the reciprocal
                # is finite (host patches the output row)
                nc.vector.tensor_copy(zsb[64:65, 511:512], ones32[:, 0:1])
            zsbs.append(zsb)
        for c4 in range(4):
            psts = []
            for h01 in range(2):
                pst1 = t_pool.tile([128, 65], F32, tag="tps", bufs=2,
                                   name=f"pst1_{pr}_{h01}_{j}_{c4}")
                nc.tensor.transpose(
                    pst1[:], zsbs[h01][:, 128 * c4:128 * (c4 + 1)],
                    eye[0:65, 0:65],
                )
                psts.append(pst1)
            zns = []
            for h01 in range(2):
                rcol = rcpool.tile([128, 1], F32, tag="rc")
                nc.vector.reciprocal(rcol[:], psts[h01][:, 64:65])
                zn = znpool.tile([128, 64], F32, tag="zn")
                nc.vector.tensor_scalar_mul(zn[:], psts[h01][:, 0:64],
                                            rcol[:])
                zns.append(zn)
            for h01 in range(2):
                rows = slice(64 * h01, 64 * (h01 + 1))
                pst2 = t_pool.tile([64, 128], F32, tag="tps", bufs=2,
                                   name=f"pst2_{pr}_{h01}_{j}_{c4}")
                nc.tensor.transpose(pst2[:], zns[h01][:], eye[:])
                nc.vector.tensor_copy(
                    tiles[("zp", pr, 4 * j + c4)][rows, :], pst2[:]
                )

    def proj(j, ps_pool):
        for c4 in range(4):
            ck = 4 * j + c4
            for mt in range(MT):
                pp = ps_pool.tile([128, 512], F32, tag="tps", bufs=2,
                                  name=f"prps{ck}_{mt}")
                nc.tensor.matmul(
                    pp[:], tiles[("zp", 0, ck)][:],
                    wos[0][:, 512 * mt:512 * (mt + 1)],
                    start=True, stop=False,
                )
                nc.tensor.matmul(
                    pp[:], tiles[("zp", 1, ck)][:],
                    wos[1][:, 512 * mt:512 * (mt + 1)],
                    start=False, stop=True,
                )
                osb = opool.tile([128, 512], F32, tag="osb")
                nc.scalar.copy(osb[:], pp[:])
                nc.gpsimd.dma_start(
                    outp[128 * ck:128 * (ck + 1), 512 * mt:512 * (mt + 1)],
                    osb[:],
                )

    def attn_small(pr, j, ps_pool):
        """Single-chunk [128,512] attention for short j (few kept chunks);
        round-robin over the pair's two heads, z trailing by DW slots."""
        qT, kT = qts[pr], kts[pr]
        ilist = list(range(PC - 1, 4 * j - 1, -1))
        nchunk = len(ilist)
        nslot = 2 * nchunk
        DW = min(4, nslot - 1)
        zpss = [ps_pool.tile([65, 512], F32, tag="qkvps",
                             name=f"zpss{pr}_{h01}_{j}")
                for h01 in range(2)]
        descs = []
        for idx in range(nslot + DW):
            if idx < nslot:
                h01, a = idx % 2, idx // 2
                rows = slice(64 * h01, 64 * (h01 + 1))
                i_ = ilist[a]
                tt = i_ - 4 * j
                w_ = min(512, 128 * (tt + 1))
                sps = ps_pool.tile([128, 512], F32, tag="qkvps",
                                   name=f"ssps{pr}_{h01}_{j}_{a}")
                nc.tensor.matmul(
                    sps[:, :w_],
                    kT[rows, 128 * i_:128 * (i_ + 1)],
                    qT[rows, 512 * j:512 * j + w_],
                    start=True, stop=True,
                )
                ex = expool.tile([128, 1024], DT_MM, tag="ex")
                nc.scalar.activation(ex[:, :w_], sps[:, :w_], EXP,
                                     scale=0.125)
                if tt < 4:
                    nc.vector.tensor_mul(
                        ex[:, 128 * tt:w_], ex[:, 128 * tt:w_], mask[:]
                    )
                descs.append((ex, h01, i_, w_))
            zi = idx - DW
            if 0 <= zi < nslot:
                ex, h01, i_, w_ = descs[zi]
                nc.tensor.matmul(
                    zpss[h01][:, :w_], tiles[("va", pr, h01, i_)][:],
                    ex[:, :w_],
                    start=(zi < 2), stop=(zi >= nslot - 2),
                )
        finish_pair(pr, j, zpss, ps_pool)

    def attn_big(pr, j, sc_pool, z_pool, t_pool):
        """Row-packed attention: both heads' K=64 score matmuls run
        concurrently in disjoint PE row-groups into one [128,1024] PSUM
        tile; one batched exp covers both. z matmuls trail by DW slots."""
        qT, kT = qts[pr], kts[pr]
        ilist = list(range(PC - 1, 4 * j - 1, -1))
        nslot = len(ilist)
        DW = min(6, nslot - 1)
        zpss = [z_pool.tile([65, 512], F32, tag="zps",
                            name=f"zps{pr}_{h01}_{j}")
                for h01 in range(2)]
        descs = []
        for idx in range(nslot + DW):
            if idx < nslot:
                i_ = ilist[idx]
                tt = i_ - 4 * j
                w_ = min(512, 128 * (tt + 1))
                sps = sc_pool.tile([128, 1024], F32, tag="scps")
                nc.tensor.matmul(
                    sps[:, :w_],
                    kT[0:64, 128 * i_:128 * (i_ + 1)],
                    qT[0:64, 512 * j:512 * j + w_],
                    start=True, stop=True,
                )
                nc.tensor.matmul(
                    sps[:, 512:512 + w_],
                    kT[64:128, 128 * i_:128 * (i_ + 1)],
                    qT[64:128, 512 * j:512 * j + w_],
                    start=True, stop=True,
                )
                ex = expool.tile([128, 1024], DT_MM, tag="ex")
                if w_ == 512:
                    nc.scalar.activation(ex[:], sps[:], EXP, scale=0.125)
                else:
                    nc.scalar.activation(ex[:, :w_], sps[:, :w_], EXP,
                                         scale=0.125)
                    nc.scalar.activation(
                        ex[:, 512:512 + w_], sps[:, 512:512 + w_], EXP,
                        scale=0.125,
                    )
                if tt < 4:
                    for off in (0, 512):
                        nc.vector.tensor_mul(
                            ex[:, off + 128 * tt:off + w_],
                            ex[:, off + 128 * tt:off + w_], mask[:]
                        )
                descs.append((ex, i_, w_))
            zi = idx - DW
            if 0 <= zi < nslot:
                ex, i_, w_ = descs[zi]
                nc.tensor.matmul(
                    zpss[0][:, :w_], tiles[("va", pr, 0, i_)][:],
                    ex[:, :w_],
                    start=(zi == 0), stop=(zi == nslot - 1),
                )
                nc.tensor.matmul(
                    zpss[1][:, :w_], tiles[("va", pr, 1, i_)][:],
                    ex[:, 512:512 + w_],
                    start=(zi == 0), stop=(zi == nslot - 1),
                )
        finish_pair(pr, j, zpss, t_pool)

    for pr in range(NPAIRS):
        for c4 in range(4):
            tiles[("zp", pr, 4 * (PT - 1) + c4)] = zppool.tile(
                [128, 128], DT_MM, tag="zp", name=f"zp{pr}_{4 * (PT - 1) + c4}")

    # ---- QKV projections, with the short j=3 attention interleaved ----
    with tc.tile_pool(name="wpool", bufs=6) as wpool, \
         tc.tile_pool(name="vtpool", bufs=4) as vtpool, \
         tc.tile_pool(name="ps_qkv", bufs=6, space="PSUM") as ps_qkv:
        wsb = {}
        bsb = {}
        for t in ("q", "k", "v"):
            for pr in range(NPAIRS):
                wt = wpool.tile([128, MK * 128], DT_MM, tag="w")
                nc.scalar.dma_start(
                    wt.rearrange("p (k f) -> p k f", k=MK),
                    aps[f"w{t}"][pr].rearrange("k p f -> p k f"),
                )
                wsb[(t, pr)] = wt
                bt = consts.tile([1, 128], DT_MM, tag=f"b{t}{pr}")
                nc.sync.dma_start(bt[:], aps[f"b{t}"][pr])
                bsb[(t, pr)] = bt
        for pr in range(NPAIRS):
            qT = qkpool.tile([128, 2048], DT_MM, tag="qT", name=f"qT{pr}")
            kT = qkpool.tile([128, 2048], DT_MM, tag="kT", name=f"kT{pr}")
            qts[pr], kts[pr] = qT, kT
            # v first, using only 2 PSUM slots so the q/k projections can
            # overlap the DVE-paced v-transpose section
            for j4a in range(0, PT, 2):
                w = wsb[("v", pr)]
                pss = [ps_qkv.tile([128, 512], F32, tag="qkvps",
                                   name=f"qkvps_v{pr}{j4a + d}")
                       for d in range(2)]
                for mk in range(MK):
                    for d in range(2):
                        nc.tensor.matmul(
                            pss[d][:],
                            w[:, 128 * mk:128 * (mk + 1)],
                            xsb[mk][:, 512 * (j4a + d):512 * (j4a + d + 1)],
                            start=(mk == 0), stop=False,
                        )
                for d in range(2):
                    nc.tensor.matmul(
                        pss[d][:], bsb[("v", pr)][:],
                        ones_mm[:], start=False, stop=True,
                    )
                for d in range(2):
                    j4 = j4a + d
                    ps = pss[d]
                    # v^T slice + ones row, PE-transposed into v_aug
                    # chunks [128 pk, 65] (col 64 = ones for denoms)
                    for h01 in range(2):
                        vts = vtpool.tile([65, 512], F32, tag="vT")
                        nc.gpsimd.tensor_copy(vts[64:65, :], ones32[:])
                        nc.vector.tensor_copy(
                            vts[0:64, :], ps[64 * h01:64 * (h01 + 1), :]
                        )
                        for c4 in range(4):
                            pst = ps_qkv.tile([128, 65], F32, tag="qkvps")
                            nc.tensor.transpose(
                                pst[:], vts[:, 128 * c4:128 * (c4 + 1)],
                                eye[0:65, 0:65],
                            )
                            va = vapool.tile([128, 65], DT_MM, tag="va")
                            nc.vector.tensor_copy(va[:], pst[:])
                            tiles[("va", pr, h01, 4 * j4 + c4)] = va
            for t, dest in (("q", qT), ("k", kT)):
                w = wsb[(t, pr)]
                # one LDWEIGHTS per m-chunk feeds 4 accumulating p-tiles
                pss = [ps_qkv.tile([128, 512], F32, tag="qkvps",
                                   name=f"qkvps_{t}{pr}{j4}")
                       for j4 in range(PT)]
                for mk in range(MK):
                    for j4 in range(PT):
                        nc.tensor.matmul(
                            pss[j4][:],
                            w[:, 128 * mk:128 * (mk + 1)],
                            xsb[mk][:, 512 * j4:512 * (j4 + 1)],
                            start=(mk == 0), stop=False,
                        )
                for j4 in range(PT):
                    nc.tensor.matmul(
                        pss[j4][:], bsb[(t, pr)][:],
                        ones_mm[:], start=False, stop=True,
                    )
                for j4 in range(PT):
                    nc.vector.tensor_copy(
                        dest[:, 512 * j4:512 * (j4 + 1)], pss[j4][:]
                    )
            # short j=PT-1 attention for this pair, hidden in the qkv stream
            attn_small(pr, PT - 1, ps_qkv)
        proj(PT - 1, ps_qkv)

    # ---- deep-pipelined attention for the remaining j ----
    with tc.tile_pool(name="ps_sc", bufs=2, space="PSUM") as ps_sc, \
         tc.tile_pool(name="ps_z", bufs=2, space="PSUM") as ps_z, \
         tc.tile_pool(name="ps_t", bufs=2, space="PSUM") as ps_t:
        for j in range(PT - 2, -1, -1):
            for pr in range(NPAIRS):
                for c4 in range(4):
                    tiles[("zp", pr, 4 * j + c4)] = zppool.tile(
                        [128, 128], DT_MM, tag="zp",
                        name=f"zp{pr}_{4 * j + c4}")
            for pr in range(NPAIRS):
                attn_big(pr, j, ps_sc, ps_z, ps_t)
            proj(j, ps_t)


def _build():
    if DT_MODE in _BUILT:
        return _BUILT[DT_MODE]
    from contextlib import ExitStack

    nc = bacc.Bacc("TRN2", target_bir_lowering=False, debug=False)
    aps = {
        "xT": nc.dram_tensor("xT", [M, P], DT_MM, kind="ExternalInput").ap(),
        "wq": nc.dram_tensor("wq", [NPAIRS, MK, 128, 128], DT_MM,
                             kind="ExternalInput").ap(),
        "wk": nc.dram_tensor("wk", [NPAIRS, MK, 128, 128], DT_MM,
                             kind="ExternalInput").ap(),
        "wv": nc.dram_tensor("wv", [NPAIRS, MK, 128, 128], DT_MM,
                             kind="ExternalInput").ap(),
        "wo": nc.dram_tensor("wo", [NPAIRS, 128, 1024], DT_MM,
                             kind="ExternalInput").ap(),
        "bq": nc.dram_tensor("bq", [NPAIRS, 1, 128], DT_MM,
                             kind="ExternalInput").ap(),
        "bk": nc.dram_tensor("bk", [NPAIRS, 1, 128], DT_MM,
                             kind="ExternalInput").ap(),
        "bv": nc.dram_tensor("bv", [NPAIRS, 1, 128], DT_MM,
                             kind="ExternalInput").ap(),
        "eye": nc.dram_tensor("eye", [128, 128], F32,
                              kind="ExternalInput").ap(),
        "mask": nc.dram_tensor("mask", [128, 128], DT_MM,
                               kind="ExternalInput").ap(),
        "outp": nc.dram_tensor("outp", [P, M], F32, kind="ExternalOutput").ap(),
    }
    with tile.TileContext(nc) as tc:
        with ExitStack() as ctx:
            _emit(nc, tc, aps, ctx)
    nc.compile()
    _BUILT[DT_MODE] = nc
    return nc


def _host_inputs(x, kq, kk, kv, ko, bq, bk, bv):
    xT = np.ascontiguousarray(x.transpose(0, 2, 1)).astype(NP_MM)  # [B, M, P]
    eye = np.eye(128, dtype=np.float32)
    # keep iff pq < pk; block mask[r(pk), c(pq)] = 1 if c < r
    mask = np.tril(np.ones((128, 128), np.float32), k=-1).astype(NP_MM)
    in_maps = []
    for c in range(NCORES):
        b, k4 = divmod(c, 4)
        heads = [4 * k4 + i for i in range(HPC)]

        def pairw(kern):
            # [NPAIRS, MK, 128, 128] lhsT chunks
            out = np.empty((NPAIRS, MK, 128, 128), NP_MM)
            for pr in range(NPAIRS):
                pairm = np.concatenate(
                    [kern[heads[2 * pr]], kern[heads[2 * pr + 1]]], axis=1
                )  # [1024, 128]
                out[pr] = pairm.reshape(MK, 128, 128).astype(NP_MM)
            return out

        def pairb(bias):
            out = np.empty((NPAIRS, 1, 128), NP_MM)
            for pr in range(NPAIRS):
                out[pr, 0] = np.concatenate(
                    [bias[heads[2 * pr]], bias[heads[2 * pr + 1]]]
                ).astype(NP_MM)
            return out

        wo = np.empty((NPAIRS, 128, 1024), NP_MM)
        for pr in range(NPAIRS):
            wo[pr] = np.concatenate(
                [ko[heads[2 * pr]], ko[heads[2 * pr + 1]]], axis=0
            ).astype(NP_MM)

        in_maps.append({
            "xT": xT[b],
            "wq": pairw(kq), "wk": pairw(kk), "wv": pairw(kv),
            "wo": wo,
            "bq": pairb(bq), "bk": pairb(bk), "bv": pairb(bv),
            "eye": eye, "mask": mask,
        })
    return in_maps


def kernel(x, kernel_query, kernel_key, kernel_value, kernel_out,
           bias_query, bias_key, bias_value, bias_out, _trace=False):
    x = np.asarray(x, np.float32)
    kq = np.asarray(kernel_query, np.float32)
    kk = np.asarray(kernel_key, np.float32)
    kv = np.asarray(kernel_value, np.float32)
    ko = np.asarray(kernel_out, np.float32)
    bq = np.asarray(bias_query, np.float32)
    bk = np.asarray(bias_key, np.float32)
    bv = np.asarray(bias_value, np.float32)
    bo = np.asarray(bias_out, np.float32)

    nc = _build()
    in_maps = _host_inputs(x, kq, kk, kv, ko, bq, bk, bv)
    res = bass_utils.run_bass_kernel_spmd(
        nc, in_maps, core_ids=list(range(NCORES)), trace=_trace
    )
    out = np.zeros((B, P, M), np.float32)
    for c in range(NCORES):
        out[c // 4] += res.results[c]["outp"]
    out += bo[None, None, :]

    # patch fully-masked query row P-1: uniform attention = mean_k v
    for b in range(B):
        xbar = x[b].mean(axis=0, dtype=np.float64)  # [M]
        row = np.zeros(M, np.float64)
        for n in range(N):
            zrow = xbar @ kv[n].astype(np.float64) + bv[n].astype(np.float64)
            row += zrow @ ko[n].astype(np.float64)
        out[b, P - 1, :] = (row + bo.astype(np.float64)).astype(np.float32)

    if _trace:
        kernel._last_result = res
    return out


# revision 35
# speedup vs baseline: 1.0042x; 1.0042x over previous
"""Trainium2 Bass kernel for multi-head attention (B=2, P=2048, M=1024, N=16, H=64).

Sharding: 8 cores = 2 batches x 4 head-groups. Core c handles batch c//4,
heads [4*(c%4), 4*(c%4)+4). Each core computes its heads' attention and the
partial output projection; the host sums partials across the 4 cores of each
batch.

Device algorithm (per core; matmul dtype selectable bf16/fp32r):
  - q^T,k^T,v^T [h', p] via projections with x^T as the moving operand,
    head-pairs concatenated to fill 128 partitions; bias added via K=1 matmul.
    One weight load feeds 4 accumulating p-tiles (LDWEIGHTS amortized).
  - scores^T [pk, pq] per head; strictly-lower-triangular keep mask (pq < pk)
    exploited by skipping fully-masked tiles and narrowing partial ones.
    Two pk-chunks of scores land in one [128,1024] PSUM tile so a single
    ScalarE exp instruction covers both (amortizes ACT fixed overhead).
  - v transposed head-wise on the PE with an appended ones row, so the z
    matmul (z_aug^T = v_aug^T @ exp^T) also yields the softmax denominators.
  - z_aug^T is PE-transposed to [pq, h] layout where the denominator is a
    per-partition scalar: reciprocal + tensor_scalar normalize, then
    PE-transposed back and head-pairs packed to K=128 for the output
    projection, which accumulates both pairs in PSUM. This per-unit work is
    interleaved with the attention stream to keep the PE fed while ScalarE
    runs exp.
  - The fully-masked query row P-1 (softmax of all -1e10 = uniform) is
    patched analytically on the host.
"""
import os
import sys

import numpy as np

if "/opt/trn_rl_repo" not in sys.path:
    sys.path.insert(0, "/opt/trn_rl_repo")

import concourse.bacc as bacc
import concourse.tile as tile
from concourse import mybir
from concourse import bass_utils
import ml_dtypes

B, P, M, N, H = 2, 2048, 1024, 16, 64
NCORES = 8
HPC = 4          # heads per core
NPAIRS = 2       # head pairs per core
MK = M // 128    # 8 contraction chunks for projections
PT = P // 512    # 4 free-dim tiles of 512 over sequence
PC = P // 128    # 16 partition chunks over sequence
MT = M // 512    # 2 output m-tiles

F32 = mybir.dt.float32
F32R = mybir.dt.float32r
BF16 = mybir.dt.bfloat16
EXP = mybir.ActivationFunctionType.Exp
MULT = mybir.AluOpType.mult

DT_MODE = os.environ.get("KERNEL_DT", "bf16")   # "bf16" | "f32r"
DT_MM = BF16 if DT_MODE == "bf16" else F32R
NP_MM = ml_dtypes.bfloat16 if DT_MODE == "bf16" else np.float32

_BUILT = {}


def _emit(nc, tc, aps, ctx):
    xT = aps["xT"]          # [1024, 2048]
    outp = aps["outp"]      # [2048, 1024]

    consts = ctx.enter_context(tc.tile_pool(name="consts", bufs=1))
    xpool = ctx.enter_context(tc.tile_pool(name="xpool", bufs=MK))
    qkpool = ctx.enter_context(tc.tile_pool(name="qkpool", bufs=2))
    vapool = ctx.enter_context(tc.tile_pool(name="vapool", bufs=68))
    zppool = ctx.enter_context(tc.tile_pool(name="zppool", bufs=16))
    expool = ctx.enter_context(
        tc.tile_pool(name="expool", bufs=(9 if DT_MODE == "bf16" else 9)))
    zsbpool = ctx.enter_context(tc.tile_pool(name="zsbpool", bufs=6))
    znpool = ctx.enter_context(tc.tile_pool(name="znpool", bufs=6))
    rcpool = ctx.enter_context(tc.tile_pool(name="rcpool", bufs=8))
    opool = ctx.enter_context(tc.tile_pool(name="opool", bufs=4))

    eye = consts.tile([128, 128], F32)
    nc.sync.dma_start(eye[:], aps["eye"][:])
    mask = consts.tile([128, 128], DT_MM)
    nc.sync.dma_start(mask[:], aps["mask"][:])
    ones32 = consts.tile([1, 512], F32)
    nc.vector.memset(ones32[:], 1.0)
    if DT_MODE == "bf16":
        ones_mm = consts.tile([1, 512], BF16)
        nc.vector.memset(ones_mm[:], 1.0)
    else:
        ones_mm = consts.tile([1, 512], F32R)
        nc.vector.tensor_copy(ones_mm[:], ones32[:])
    wos = []
    for pr in range(NPAIRS):
        wot = consts.tile([128, 1024], DT_MM, tag=f"wo{pr}", name=f"wo{pr}")
        nc.sync.dma_start(wot[:], aps["wo"][pr])
        wos.append(wot)

    # x^T chunks [128 m, 2048 p]
    xsb = []
    for k in range(MK):
        xt = xpool.tile([128, 2048], DT_MM, tag="x")
        nc.sync.dma_start(xt[:], xT[128 * k:128 * (k + 1), :])
        xsb.append(xt)

    tiles = {}
    qts, kts = {}, {}

    def finish_pair(pr, j, zpss, t_pool):
        """Copy both heads' z_aug^T out of PSUM, then normalize in pq-space
        with the two heads' transpose chains interleaved (hides the
        PE->DVE->PE latency of each chain)."""
        zsbs = []
        for h01 in range(2):
            zsb = zsbpool.tile([65, 512], F32, tag="z",
                               name=f"zsb{pr}_{h01}_{j}")
            nc.vector.tensor_copy(zsb[:], zpss[h01][:])
            if j == PT - 1:
                # fully-masked query row P-1: denom 0 -> 1 so the reciprocal
                # is finite (host patches the output row)
                nc.vector.tensor_copy(zsb[64:65, 511:512], ones32[:, 0:1])
            zsbs.append(zsb)
        for c4 in range(4):
            psts = []
            for h01 in range(2):
                pst1 = t_pool.tile([128, 65], F32, tag="tps", bufs=2,
                                   name=f"pst1_{pr}_{h01}_{j}_{c4}")
                nc.tensor.transpose(
                    pst1[:], zsbs[h01][:, 128 * c4:128 * (c4 + 1)],
                    eye[0:65, 0:65],
                )
                psts.append(pst1)
            zns = []
            for h01 in range(2):
                rcol = rcpool.tile([128, 1], F32, tag="rc")
                nc.vector.reciprocal(rcol[:], psts[h01][:, 64:65])
                zn = znpool.tile([128, 64], F32, tag="zn")
                nc.vector.tensor_scalar_mul(zn[:], psts[h01][:, 0:64],
                                            rcol[:])
                zns.append(zn)
            for h01 in range(2):
                rows = slice(64 * h01, 64 * (h01 + 1))
                pst2 = t_pool.tile([64, 128], F32, tag="tps", bufs=2,
                                   name=f"pst2_{pr}_{h01}_{j}_{c4}")
                nc.tensor.transpose(pst2[:], zns[h01][:], eye[:])
                nc.vector.tensor_copy(
                    tiles[("zp", pr, 4 * j + c4)][rows, :], pst2[:]
                )

    def proj(j, ps_pool):
        for c4 in range(4):
            ck = 4 * j + c4
            for mt in range(MT):
                pp = ps_pool.tile([128, 512], F32, tag="tps", bufs=2,
                                  name=f"prps{ck}_{mt}")
                nc.tensor.matmul(
                    pp[:], tiles[("zp", 0, ck)][:],
                    wos[0][:, 512 * mt:512 * (mt + 1)],
                    start=True, stop=False,
                )
                nc.tensor.matmul(
                    pp[:], tiles[("zp", 1, ck)][:],
                    wos[1][:, 512 * mt:512 * (mt + 1)],
                    start=False, stop=True,
                )
                osb = opool.tile([128, 512], F32, tag="osb")
                nc.scalar.copy(osb[:], pp[:])
                nc.gpsimd.dma_start(
                    outp[128 * ck:128 * (ck + 1), 512 * mt:512 * (mt + 1)],
                    osb[:],
                )

    def attn_small(pr, j, ps_pool):
        """Single-chunk [128,512] attention for short j (few kept chunks);
        round-robin over the pair's two heads, z trailing by DW slots."""
        qT, kT = qts[pr], kts[pr]
        ilist = list(range(PC - 1, 4 * j - 1, -1))
        nchunk = len(ilist)
        nslot = 2 * nchunk
        DW = min(4, nslot - 1)
        zpss = [ps_pool.tile([65, 512], F32, tag="qkvps",
                             name=f"zpss{pr}_{h01}_{j}")
                for h01 in range(2)]
        descs = []
        for idx in range(nslot + DW):
            if idx < nslot:
                h01, a = idx % 2, idx // 2
                rows = slice(64 * h01, 64 * (h01 + 1))
                i_ = ilist[a]
                tt = i_ - 4 * j
                w_ = min(512, 128 * (tt + 1))
                sps = ps_pool.tile([128, 512], F32, tag="qkvps",
                                   name=f"ssps{pr}_{h01}_{j}_{a}")
                nc.tensor.matmul(
                    sps[:, :w_],
                    kT[rows, 128 * i_:128 * (i_ + 1)],
                    qT[rows, 512 * j:512 * j + w_],
                    start=True, stop=True,
                )
                ex = expool.tile([128, 1024], DT_MM, tag="ex")
                nc.scalar.activation(ex[:, :w_], sps[:, :w_], EXP,
                                     scale=0.125)
                if tt < 4:
                    nc.vector.tensor_mul(
                        ex[:, 128 * tt:w_], ex[:, 128 * tt:w_], mask[:]
                    )
                descs.append((ex, h01, i_, w_))
            zi = idx - DW
            if 0 <= zi < nslot:
                ex, h01, i_, w_ = descs[zi]
                nc.tensor.matmul(
                    zpss[h01][:, :w_], tiles[("va", pr, h01, i_)][:],
                    ex[:, :w_],
                    start=(zi < 2), stop=(zi >= nslot - 2),
                )
        finish_pair(pr, j, zpss, ps_pool)

    def attn_big(pr, j, sc_pool, z_pool, t_pool):
        """Row-packed attention: both heads' K=64 score matmuls run
        concurrently in disjoint PE row-groups into one [128,1024] PSUM
        tile; one batched exp covers both. z matmuls trail by DW slots."""
        qT, kT = qts[pr], kts[pr]
        ilist = list(range(PC - 1, 4 * j - 1, -1))
        nslot = len(ilist)
        DW = min(6, nslot - 1)
        zpss = [z_pool.tile([65, 512], F32, tag="zps",
                            name=f"zps{pr}_{h01}_{j}")
                for h01 in range(2)]
        descs = []
        for idx in range(nslot + DW):
            if idx < nslot:
                i_ = ilist[idx]
                tt = i_ - 4 * j
                w_ = min(512, 128 * (tt + 1))
                sps = sc_pool.tile([128, 1024], F32, tag="scps")
                nc.tensor.matmul(
                    sps[:, :w_],
                    kT[0:64, 128 * i_:128 * (i_ + 1)],
                    qT[0:64, 512 * j:512 * j + w_],
                    start=True, stop=True,
                )
                nc.tensor.matmul(
                    sps[:, 512:512 + w_],
                    kT[64:128, 128 * i_:128 * (i_ + 1)],
                    qT[64:128, 512 * j:512 * j + w_],
                    start=True, stop=True,
                )
                ex = expool.tile([128, 1024], DT_MM, tag="ex")
                if w_ == 512:
                    nc.scalar.activation(ex[:], sps[:], EXP, scale=0.125)
                else:
                    nc.scalar.activation(ex[:, :w_], sps[:, :w_], EXP,
                                         scale=0.125)
                    nc.scalar.activation(
                        ex[:, 512:512 + w_], sps[:, 512:512 + w_], EXP,
                        scale=0.125,
                    )
                if tt < 4:
                    for off in (0, 512):
                        nc.vector.tensor_mul(
                            ex[:, off + 128 * tt:off + w_],
                            ex[:, off + 128 * tt:off + w_], mask[:]
                        )
                descs.append((ex, i_, w_))
            zi = idx - DW
            if 0 <= zi < nslot:
                ex, i_, w_ = descs[zi]
                nc.tensor.matmul(
                    zpss[0][:, :w_], tiles[("va", pr, 0, i_)][:],
                    ex[:, :w_],
                    start=(zi == 0), stop=(zi == nslot - 1),
                )
                nc.tensor.matmul(
                    zpss[1][:, :w_], tiles[("va", pr, 1, i_)][:],
                    ex[:, 512:512 + w_],
                    start=(zi == 0), stop=(zi == nslot - 1),
                )
        finish_pair(pr, j, zpss, t_pool)

    for pr in range(NPAIRS):
        for c4 in range(4):
            tiles[("zp", pr, 4 * (PT - 1) + c4)] = zppool.tile(
                [128, 128], DT_MM, tag="zp", name=f"zp{pr}_{4 * (PT - 1) + c4}")

    # ---- QKV projections, with the short j=3 attention interleaved ----
    with tc.tile_pool(name="wpool", bufs=6) as wpool, \
         tc.tile_pool(name="vtpool", bufs=4) as vtpool, \
         tc.tile_pool(name="ps_qkv", bufs=6, space="PSUM") as ps_qkv:
        wsb = {}
        bsb = {}
        for pr in range(NPAIRS):
            for t in ("v", "q", "k"):
                bt = consts.tile([1, 128], DT_MM, tag=f"b{t}{pr}")
                nc.scalar.dma_start(bt[:], aps[f"b{t}"][pr])
                bsb[(t, pr)] = bt
        for pr in range(NPAIRS):
            for t in ("v", "q", "k"):
                wt = wpool.tile([128, MK * 128], DT_MM, tag="w",
                                name=f"w_{t}{pr}")
                nc.scalar.dma_start(
                    wt.rearrange("p (k f) -> p k f", k=MK),
                    aps[f"w{t}"][pr].rearrange("k p f -> p k f"),
                )
                wsb[(t, pr)] = wt
        for pr in range(NPAIRS):
            qT = qkpool.tile([128, 2048], DT_MM, tag="qT", name=f"qT{pr}")
            kT = qkpool.tile([128, 2048], DT_MM, tag="kT", name=f"kT{pr}")
            qts[pr], kts[pr] = qT, kT
            # v first, using only 2 PSUM slots so the q/k projections can
            # overlap the DVE-paced v-transpose section
            for j4a in range(0, PT, 2):
                w = wsb[("v", pr)]
                pss = [ps_qkv.tile([128, 512], F32, tag="qkvps",
                                   name=f"qkvps_v{pr}{j4a + d}")
                       for d in range(2)]
                for mk in range(MK):
                    for d in range(2):
                        nc.tensor.matmul(
                            pss[d][:],
                            w[:, 128 * mk:128 * (mk + 1)],
                            xsb[mk][:, 512 * (j4a + d):512 * (j4a + d + 1)],
                            start=(mk == 0), stop=False,
                        )
                for d in range(2):
                    nc.tensor.matmul(
                        pss[d][:], bsb[("v", pr)][:],
                        ones_mm[:], start=False, stop=True,
                    )
                for d in range(2):
                    j4 = j4a + d
                    ps = pss[d]
                    # v^T slice + ones row, PE-transposed into v_aug
                    # chunks [128 pk, 65] (col 64 = ones for denoms)
                    for h01 in range(2):
                        vts = vtpool.tile([65, 512], F32, tag="vT")
                        nc.gpsimd.tensor_copy(vts[64:65, :], ones32[:])
                        nc.vector.tensor_copy(
                            vts[0:64, :], ps[64 * h01:64 * (h01 + 1), :]
                        )
                        for c4 in range(4):
                            pst = ps_qkv.tile([128, 65], F32, tag="qkvps")
                            nc.tensor.transpose(
                                pst[:], vts[:, 128 * c4:128 * (c4 + 1)],
                                eye[0:65, 0:65],
                            )
                            va = vapool.tile([128, 65], DT_MM, tag="va")
                            nc.vector.tensor_copy(va[:], pst[:])
                            tiles[("va", pr, h01, 4 * j4 + c4)] = va
            for t, dest in (("q", qT), ("k", kT)):
                w = wsb[(t, pr)]
                # one LDWEIGHTS per m-chunk feeds 4 accumulating p-tiles
                pss = [ps_qkv.tile([128, 512], F32, tag="qkvps",
                                   name=f"qkvps_{t}{pr}{j4}")
                       for j4 in range(PT)]
                for mk in range(MK):
                    for j4 in range(PT):
                        nc.tensor.matmul(
                            pss[j4][:],
                            w[:, 128 * mk:128 * (mk + 1)],
                            xsb[mk][:, 512 * j4:512 * (j4 + 1)],
                            start=(mk == 0), stop=False,
                        )
                for j4 in range(PT):
                    nc.tensor.matmul(
                        pss[j4][:], bsb[(t, pr)][:],
                        ones_mm[:], start=False, stop=True,
                    )
                for j4 in range(PT):
                    nc.vector.tensor_copy(
                        dest[:, 512 * j4:512 * (j4 + 1)], pss[j4][:]
                    )
            # short j=PT-1 attention for this pair, hidden in the qkv stream
            attn_small(pr, PT - 1, ps_qkv)
        proj(PT - 1, ps_qkv)

    # ---- deep-pipelined attention for the remaining j ----
    with tc.tile_pool(name="ps_sc", bufs=2, space="PSUM") as ps_sc, \
         tc.tile_pool(name="ps_z", bufs=2, space="PSUM") as ps_z, \
         tc.tile_pool(name="ps_t", bufs=2, space="PSUM") as ps_t:
        for j in range(PT - 2, -1, -1):
            for pr in range(NPAIRS):
                for c4 in range(4):
                    tiles[("zp", pr, 4 * j + c4)] = zppool.tile(
                        [128, 128], DT_MM, tag="zp",
                        name=f"zp{pr}_{4 * j + c4}")
            for pr in range(NPAIRS):
                attn_big(pr, j, ps_sc, ps_z, ps_t)
            proj(j, ps_t)


def _build():
    if DT_MODE in _BUILT:
        return _BUILT[DT_MODE]
    from contextlib import ExitStack

    nc = bacc.Bacc("TRN2", target_bir_lowering=False, debug=False)
    aps = {
        "xT": nc.dram_tensor("xT", [M, P], DT_MM, kind="ExternalInput").ap(),
        "wq": nc.dram_tensor("wq", [NPAIRS, MK, 128, 128], DT_MM,
                             kind="ExternalInput").ap(),
        "wk": nc.dram_tensor("wk", [NPAIRS, MK, 128, 128], DT_MM,
                             kind="ExternalInput").ap(),
        "wv": nc.dram_tensor("wv", [NPAIRS, MK, 128, 128], DT_MM,
                             kind="ExternalInput").ap(),
        "wo": nc.dram_tensor("wo", [NPAIRS, 128, 1024], DT_MM,
                             kind="ExternalInput").ap(),
        "bq": nc.dram_tensor("bq", [NPAIRS, 1, 128], DT_MM,
                             kind="ExternalInput").ap(),
        "bk": nc.dram_tensor("bk", [NPAIRS, 1, 128], DT_MM,
                             kind="ExternalInput").ap(),
        "bv": nc.dram_tensor("bv", [NPAIRS, 1, 128], DT_MM,
                             kind="ExternalInput").ap(),
        "eye": nc.dram_tensor("eye", [128, 128], F32,
                              kind="ExternalInput").ap(),
        "mask": nc.dram_tensor("mask", [128, 128], DT_MM,
                               kind="ExternalInput").ap(),
        "outp": nc.dram_tensor("outp", [P, M], F32, kind="ExternalOutput").ap(),
    }
    with tile.TileContext(nc) as tc:
        with ExitStack() as ctx:
            _emit(nc, tc, aps, ctx)
    nc.compile()
    _BUILT[DT_MODE] = nc
    return nc


def _host_inputs(x, kq, kk, kv, ko, bq, bk, bv):
    xT = np.ascontiguousarray(x.transpose(0, 2, 1)).astype(NP_MM)  # [B, M, P]
    eye = np.eye(128, dtype=np.float32)
    # keep iff pq < pk; block mask[r(pk), c(pq)] = 1 if c < r
    mask = np.tril(np.ones((128, 128), np.float32), k=-1).astype(NP_MM)
    in_maps = []
    for c in range(NCORES):
        b, k4 = divmod(c, 4)
        heads = [4 * k4 + i for i in range(HPC)]

        def pairw(kern):
            # [NPAIRS, MK, 128, 128] lhsT chunks
            out = np.empty((NPAIRS, MK, 128, 128), NP_MM)
            for pr in range(NPAIRS):
                pairm = np.concatenate(
                    [kern[heads[2 * pr]], kern[heads[2 * pr + 1]]], axis=1
                )  # [1024, 128]
                out[pr] = pairm.reshape(MK, 128, 128).astype(NP_MM)
            return out

        def pairb(bias):
            out = np.empty((NPAIRS, 1, 128), NP_MM)
            for pr in range(NPAIRS):
                out[pr, 0] = np.concatenate(
                    [bias[heads[2 * pr]], bias[heads[2 * pr + 1]]]
                ).astype(NP_MM)
            return out

        wo = np.empty((NPAIRS, 128, 1024), NP_MM)
        for pr in range(NPAIRS):
            wo[pr] = np.concatenate(
                [ko[heads[2 * pr]], ko[heads[2 * pr + 1]]], axis=0
            ).astype(NP_MM)

        in_maps.append({
            "xT": xT[b],
            "wq": pairw(kq), "wk": pairw(kk), "wv": pairw(kv),
            "wo": wo,
            "bq": pairb(bq), "bk": pairb(bk), "bv": pairb(bv),
            "eye": eye, "mask": mask,
        })
    return in_maps


def kernel(x, kernel_query, kernel_key, kernel_value, kernel_out,
           bias_query, bias_key, bias_value, bias_out, _trace=False):
    x = np.asarray(x, np.float32)
    kq = np.asarray(kernel_query, np.float32)
    kk = np.asarray(kernel_key, np.float32)
    kv = np.asarray(kernel_value, np.float32)
    ko = np.asarray(kernel_out, np.float32)
    bq = np.asarray(bias_query, np.float32)
    bk = np.asarray(bias_key, np.float32)
    bv = np.asarray(bias_value, np.float32)
    bo = np.asarray(bias_out, np.float32)

    nc = _build()
    in_maps = _host_inputs(x, kq, kk, kv, ko, bq, bk, bv)
    res = bass_utils.run_bass_kernel_spmd(
        nc, in_maps, core_ids=list(range(NCORES)), trace=_trace
    )
    out = np.zeros((B, P, M), np.float32)
    for c in range(NCORES):
        out[c // 4] += res.results[c]["outp"]
    out += bo[None, None, :]

    # patch fully-masked query row P-1: uniform attention = mean_k v
    for b in range(B):
        xbar = x[b].mean(axis=0, dtype=np.float64)  # [M]
        row = np.zeros(M, np.float64)
        for n in range(N):
            zrow = xbar @ kv[n].astype(np.float64) + bv[n].astype(np.float64)
            row += zrow @ ko[n].astype(np.float64)
        out[b, P - 1, :] = (row + bo.astype(np.float64)).astype(np.float32)

    if _trace:
        kernel._last_result = res
    return out


# revision 36
# speedup vs baseline: 1.0822x; 1.0777x over previous
"""Trainium2 Bass kernel for multi-head attention (B=2, P=2048, M=1024, N=16, H=64).

Sharding: 8 cores = 2 batches x 4 head-groups. Core c handles batch c//4,
heads [4*(c%4), 4*(c%4)+4). Each core computes its heads' attention and the
partial output projection; the host sums partials across the 4 cores of each
batch.

Device algorithm (per core; matmul dtype selectable bf16/fp32r):
  - q^T,k^T,v^T [h', p] via projections with x^T as the moving operand,
    head-pairs concatenated to fill 128 partitions; bias added via K=1 matmul.
    One weight load feeds 4 accumulating p-tiles (LDWEIGHTS amortized).
  - scores^T [pk, pq] per head; strictly-lower-triangular keep mask (pq < pk)
    exploited by skipping fully-masked tiles and narrowing partial ones.
    Two pk-chunks of scores land in one [128,1024] PSUM tile so a single
    ScalarE exp instruction covers both (amortizes ACT fixed overhead).
  - v transposed head-wise on the PE with an appended ones row, so the z
    matmul (z_aug^T = v_aug^T @ exp^T) also yields the softmax denominators.
  - z_aug^T is PE-transposed to [pq, h] layout where the denominator is a
    per-partition scalar: reciprocal + tensor_scalar normalize, then
    PE-transposed back and head-pairs packed to K=128 for the output
    projection, which accumulates both pairs in PSUM. This per-unit work is
    interleaved with the attention stream to keep the PE fed while ScalarE
    runs exp.
  - The fully-masked query row P-1 (softmax of all -1e10 = uniform) is
    patched analytically on the host.
"""
import os
import sys

import numpy as np

if "/opt/trn_rl_repo" not in sys.path:
    sys.path.insert(0, "/opt/trn_rl_repo")

import concourse.bacc as bacc
import concourse.tile as tile
from concourse import mybir
from concourse import bass_utils
import ml_dtypes

B, P, M, N, H = 2, 2048, 1024, 16, 64
NCORES = 8
HPC = 4          # heads per core
NPAIRS = 2       # head pairs per core
MK = M // 128    # 8 contraction chunks for projections
PT = P // 512    # 4 free-dim tiles of 512 over sequence
PC = P // 128    # 16 partition chunks over sequence
MT = M // 512    # 2 output m-tiles

F32 = mybir.dt.float32
F32R = mybir.dt.float32r
BF16 = mybir.dt.bfloat16
EXP = mybir.ActivationFunctionType.Exp
MULT = mybir.AluOpType.mult

DT_MODE = os.environ.get("KERNEL_DT", "bf16")   # "bf16" | "f32r"
DT_MM = BF16 if DT_MODE == "bf16" else F32R
NP_MM = ml_dtypes.bfloat16 if DT_MODE == "bf16" else np.float32

_BUILT = {}


def _emit(nc, tc, aps, ctx):
    xT = aps["xT"]          # [1024, 2048]
    outp = aps["outp"]      # [2048, 1024]

    consts = ctx.enter_context(tc.tile_pool(name="consts", bufs=1))
    xpool = ctx.enter_context(tc.tile_pool(name="xpool", bufs=MK))
    qkpool = ctx.enter_context(tc.tile_pool(name="qkpool", bufs=2))
    vapool = ctx.enter_context(tc.tile_pool(name="vapool", bufs=68))
    zppool = ctx.enter_context(tc.tile_pool(name="zppool", bufs=16))
    expool = ctx.enter_context(
        tc.tile_pool(name="expool", bufs=(9 if DT_MODE == "bf16" else 9)))
    zsbpool = ctx.enter_context(tc.tile_pool(name="zsbpool", bufs=6))
    znpool = ctx.enter_context(tc.tile_pool(name="znpool", bufs=6))
    rcpool = ctx.enter_context(tc.tile_pool(name="rcpool", bufs=8))
    opool = ctx.enter_context(tc.tile_pool(name="opool", bufs=4))

    eye = consts.tile([128, 128], F32)
    nc.sync.dma_start(eye[:], aps["eye"][:])
    mask = consts.tile([128, 128], DT_MM)
    nc.sync.dma_start(mask[:], aps["mask"][:])
    ones32 = consts.tile([1, 512], F32)
    nc.vector.memset(ones32[:], 1.0)
    if DT_MODE == "bf16":
        ones_mm = consts.tile([1, 512], BF16)
        nc.vector.memset(ones_mm[:], 1.0)
    else:
        ones_mm = consts.tile([1, 512], F32R)
        nc.vector.tensor_copy(ones_mm[:], ones32[:])
    wos = []
    for pr in range(NPAIRS):
        wot = consts.tile([128, 1024], DT_MM, tag=f"wo{pr}", name=f"wo{pr}")
        nc.sync.dma_start(wot[:], aps["wo"][pr])
        wos.append(wot)

    # x^T chunks [128 m, 2048 p]
    xsb = []
    for k in range(MK):
        xt = xpool.tile([128, 2048], DT_MM, tag="x")
        nc.sync.dma_start(xt[:], xT[128 * k:128 * (k + 1), :])
        xsb.append(xt)

    tiles = {}
    qts, kts = {}, {}

    def finish_pair(pr, j, zpss, t_pool):
        """Copy both heads' z_aug^T out of PSUM, then normalize in pq-space
        with the two heads' transpose chains interleaved (hides the
        PE->DVE->PE latency of each chain)."""
        zsbs = []
        for h01 in range(2):
            zsb = zsbpool.tile([65, 512], F32, tag="z",
                               name=f"zsb{pr}_{h01}_{j}")
            nc.vector.tensor_copy(zsb[:], zpss[h01][:])
            if j == PT - 1:
                # fully-masked query row P-1: denom 0 -> 1 so the reciprocal
                # is finite (host patches the output row)
                nc.vector.tensor_copy(zsb[64:65, 511:512], ones32[:, 0:1])
            zsbs.append(zsb)
        for c4 in range(4):
            psts = []
            for h01 in range(2):
                pst1 = t_pool.tile([128, 65], F32, tag="tps", bufs=2,
                                   name=f"pst1_{pr}_{h01}_{j}_{c4}")
                nc.tensor.transpose(
                    pst1[:], zsbs[h01][:, 128 * c4:128 * (c4 + 1)],
                    eye[0:65, 0:65],
                )
                psts.append(pst1)
            zns = []
            for h01 in range(2):
                rcol = rcpool.tile([128, 1], F32, tag="rc")
                nc.vector.reciprocal(rcol[:], psts[h01][:, 64:65])
                zn = znpool.tile([128, 64], F32, tag="zn")
                nc.vector.tensor_scalar_mul(zn[:], psts[h01][:, 0:64],
                                            rcol[:])
                zns.append(zn)
            for h01 in range(2):
                rows = slice(64 * h01, 64 * (h01 + 1))
                pst2 = t_pool.tile([64, 128], F32, tag="tps", bufs=2,
                                   name=f"pst2_{pr}_{h01}_{j}_{c4}")
                nc.tensor.transpose(pst2[:], zns[h01][:], eye[:])
                nc.vector.tensor_copy(
                    tiles[("zp", pr, 4 * j + c4)][rows, :], pst2[:]
                )

    def proj(j, ps_pool):
        for c4 in range(4):
            ck = 4 * j + c4
            for mt in range(MT):
                pp = ps_pool.tile([128, 512], F32, tag="tps", bufs=2,
                                  name=f"prps{ck}_{mt}")
                nc.tensor.matmul(
                    pp[:], tiles[("zp", 0, ck)][:],
                    wos[0][:, 512 * mt:512 * (mt + 1)],
                    start=True, stop=False,
                )
                nc.tensor.matmul(
                    pp[:], tiles[("zp", 1, ck)][:],
                    wos[1][:, 512 * mt:512 * (mt + 1)],
                    start=False, stop=True,
                )
                osb = opool.tile([128, 512], F32, tag="osb")
                nc.vector.tensor_copy(osb[:], pp[:])
                nc.gpsimd.dma_start(
                    outp[128 * ck:128 * (ck + 1), 512 * mt:512 * (mt + 1)],
                    osb[:],
                )

    def attn_small(pr, j, ps_pool):
        """Single-chunk [128,512] attention for short j (few kept chunks);
        round-robin over the pair's two heads, z trailing by DW slots."""
        qT, kT = qts[pr], kts[pr]
        ilist = list(range(PC - 1, 4 * j - 1, -1))
        nchunk = len(ilist)
        nslot = 2 * nchunk
        DW = min(4, nslot - 1)
        zpss = [ps_pool.tile([65, 512], F32, tag="qkvps",
                             name=f"zpss{pr}_{h01}_{j}")
                for h01 in range(2)]
        descs = []
        for idx in range(nslot + DW):
            if idx < nslot:
                h01, a = idx % 2, idx // 2
                rows = slice(64 * h01, 64 * (h01 + 1))
                i_ = ilist[a]
                tt = i_ - 4 * j
                w_ = min(512, 128 * (tt + 1))
                sps = ps_pool.tile([128, 512], F32, tag="qkvps",
                                   name=f"ssps{pr}_{h01}_{j}_{a}")
                nc.tensor.matmul(
                    sps[:, :w_],
                    kT[rows, 128 * i_:128 * (i_ + 1)],
                    qT[rows, 512 * j:512 * j + w_],
                    start=True, stop=True,
                )
                ex = expool.tile([128, 1024], DT_MM, tag="ex")
                nc.scalar.activation(ex[:, :w_], sps[:, :w_], EXP,
                                     scale=0.125)
                if tt < 4:
                    nc.vector.tensor_mul(
                        ex[:, 128 * tt:w_], ex[:, 128 * tt:w_], mask[:]
                    )
                descs.append((ex, h01, i_, w_))
            zi = idx - DW
            if 0 <= zi < nslot:
                ex, h01, i_, w_ = descs[zi]
                nc.tensor.matmul(
                    zpss[h01][:, :w_], tiles[("va", pr, h01, i_)][:],
                    ex[:, :w_],
                    start=(zi < 2), stop=(zi >= nslot - 2),
                )
        finish_pair(pr, j, zpss, ps_pool)

    def attn_big(pr, j, sc_pool, z_pool, t_pool):
        """Row-packed attention: both heads' K=64 score matmuls run
        concurrently in disjoint PE row-groups into one [128,1024] PSUM
        tile; one batched exp covers both. z matmuls trail by DW slots."""
        qT, kT = qts[pr], kts[pr]
        ilist = list(range(PC - 1, 4 * j - 1, -1))
        nslot = len(ilist)
        DW = min(6, nslot - 1)
        zpss = [z_pool.tile([65, 512], F32, tag="zps",
                            name=f"zps{pr}_{h01}_{j}")
                for h01 in range(2)]
        descs = []
        for idx in range(nslot + DW):
            if idx < nslot:
                i_ = ilist[idx]
                tt = i_ - 4 * j
                w_ = min(512, 128 * (tt + 1))
                sps = sc_pool.tile([128, 1024], F32, tag="scps")
                nc.tensor.matmul(
                    sps[:, :w_],
                    kT[0:64, 128 * i_:128 * (i_ + 1)],
                    qT[0:64, 512 * j:512 * j + w_],
                    start=True, stop=True,
                )
                nc.tensor.matmul(
                    sps[:, 512:512 + w_],
                    kT[64:128, 128 * i_:128 * (i_ + 1)],
                    qT[64:128, 512 * j:512 * j + w_],
                    start=True, stop=True,
                )
                ex = expool.tile([128, 1024], DT_MM, tag="ex")
                if w_ == 512:
                    nc.scalar.activation(ex[:], sps[:], EXP, scale=0.125)
                else:
                    nc.scalar.activation(ex[:, :w_], sps[:, :w_], EXP,
                                         scale=0.125)
                    nc.scalar.activation(
                        ex[:, 512:512 + w_], sps[:, 512:512 + w_], EXP,
                        scale=0.125,
                    )
                if tt < 4:
                    for off in (0, 512):
                        nc.vector.tensor_mul(
                            ex[:, off + 128 * tt:off + w_],
                            ex[:, off + 128 * tt:off + w_], mask[:]
                        )
                descs.append((ex, i_, w_))
            zi = idx - DW
            if 0 <= zi < nslot:
                ex, i_, w_ = descs[zi]
                nc.tensor.matmul(
                    zpss[0][:, :w_], tiles[("va", pr, 0, i_)][:],
                    ex[:, :w_],
                    start=(zi == 0), stop=(zi == nslot - 1),
                )
                nc.tensor.matmul(
                    zpss[1][:, :w_], tiles[("va", pr, 1, i_)][:],
                    ex[:, 512:512 + w_],
                    start=(zi == 0), stop=(zi == nslot - 1),
                )
        finish_pair(pr, j, zpss, t_pool)

    for pr in range(NPAIRS):
        for c4 in range(4):
            tiles[("zp", pr, 4 * (PT - 1) + c4)] = zppool.tile(
                [128, 128], DT_MM, tag="zp", name=f"zp{pr}_{4 * (PT - 1) + c4}")

    # ---- QKV projections, with the short j=3 attention interleaved ----
    with tc.tile_pool(name="wpool", bufs=6) as wpool, \
         tc.tile_pool(name="vtpool", bufs=4) as vtpool, \
         tc.tile_pool(name="ps_qkv", bufs=6, space="PSUM") as ps_qkv:
        wsb = {}
        bsb = {}
        for pr in range(NPAIRS):
            for t in ("v", "q", "k"):
                bt = consts.tile([1, 128], DT_MM, tag=f"b{t}{pr}")
                nc.scalar.dma_start(bt[:], aps[f"b{t}"][pr])
                bsb[(t, pr)] = bt
        for pr in range(NPAIRS):
            for t in ("v", "q", "k"):
                wt = wpool.tile([128, MK * 128], DT_MM, tag="w",
                                name=f"w_{t}{pr}")
                nc.scalar.dma_start(
                    wt.rearrange("p (k f) -> p k f", k=MK),
                    aps[f"w{t}"][pr].rearrange("k p f -> p k f"),
                )
                wsb[(t, pr)] = wt
        for pr in range(NPAIRS):
            qT = qkpool.tile([128, 2048], DT_MM, tag="qT", name=f"qT{pr}")
            kT = qkpool.tile([128, 2048], DT_MM, tag="kT", name=f"kT{pr}")
            qts[pr], kts[pr] = qT, kT
            # v first, using only 2 PSUM slots so the q/k projections can
            # overlap the DVE-paced v-transpose section
            for j4a in range(0, PT, 2):
                w = wsb[("v", pr)]
                pss = [ps_qkv.tile([128, 512], F32, tag="qkvps",
                                   name=f"qkvps_v{pr}{j4a + d}")
                       for d in range(2)]
                for mk in range(MK):
                    for d in range(2):
                        nc.tensor.matmul(
                            pss[d][:],
                            w[:, 128 * mk:128 * (mk + 1)],
                            xsb[mk][:, 512 * (j4a + d):512 * (j4a + d + 1)],
                            start=(mk == 0), stop=False,
                        )
                for d in range(2):
                    nc.tensor.matmul(
                        pss[d][:], bsb[("v", pr)][:],
                        ones_mm[:], start=False, stop=True,
                    )
                for d in range(2):
                    j4 = j4a + d
                    ps = pss[d]
                    # v^T slice + ones row, PE-transposed into v_aug
                    # chunks [128 pk, 65] (col 64 = ones for denoms)
                    for h01 in range(2):
                        vts = vtpool.tile([65, 512], F32, tag="vT")
                        nc.gpsimd.tensor_copy(vts[64:65, :], ones32[:])
                        nc.scalar.copy(
                            vts[0:64, :], ps[64 * h01:64 * (h01 + 1), :]
                        )
                        for c4 in range(4):
                            pst = ps_qkv.tile([128, 65], F32, tag="qkvps")
                            nc.tensor.transpose(
                                pst[:], vts[:, 128 * c4:128 * (c4 + 1)],
                                eye[0:65, 0:65],
                            )
                            va = vapool.tile([128, 65], DT_MM, tag="va")
                            nc.vector.tensor_copy(va[:], pst[:])
                            tiles[("va", pr, h01, 4 * j4 + c4)] = va
            for t, dest in (("q", qT), ("k", kT)):
                w = wsb[(t, pr)]
                # one LDWEIGHTS per m-chunk feeds 4 accumulating p-tiles
                pss = [ps_qkv.tile([128, 512], F32, tag="qkvps",
                                   name=f"qkvps_{t}{pr}{j4}")
                       for j4 in range(PT)]
                for mk in range(MK):
                    for j4 in range(PT):
                        nc.tensor.matmul(
                            pss[j4][:],
                            w[:, 128 * mk:128 * (mk + 1)],
                            xsb[mk][:, 512 * j4:512 * (j4 + 1)],
                            start=(mk == 0), stop=False,
                        )
                for j4 in range(PT):
                    nc.tensor.matmul(
                        pss[j4][:], bsb[(t, pr)][:],
                        ones_mm[:], start=False, stop=True,
                    )
                for j4 in range(PT):
                    nc.scalar.copy(
                        dest[:, 512 * j4:512 * (j4 + 1)], pss[j4][:]
                    )
            # short j=PT-1 attention for this pair, hidden in the qkv stream
            attn_small(pr, PT - 1, ps_qkv)
        proj(PT - 1, ps_qkv)

    # ---- deep-pipelined attention for the remaining j ----
    with tc.tile_pool(name="ps_sc", bufs=2, space="PSUM") as ps_sc, \
         tc.tile_pool(name="ps_z", bufs=2, space="PSUM") as ps_z, \
         tc.tile_pool(name="ps_t", bufs=2, space="PSUM") as ps_t:
        for j in range(PT - 2, -1, -1):
            for pr in range(NPAIRS):
                for c4 in range(4):
                    tiles[("zp", pr, 4 * j + c4)] = zppool.tile(
                        [128, 128], DT_MM, tag="zp",
                        name=f"zp{pr}_{4 * j + c4}")
            for pr in range(NPAIRS):
                attn_big(pr, j, ps_sc, ps_z, ps_t)
            proj(j, ps_t)


def _build():
    if DT_MODE in _BUILT:
        return _BUILT[DT_MODE]
    from contextlib import ExitStack

    nc = bacc.Bacc("TRN2", target_bir_lowering=False, debug=False)
    aps = {
        "xT": nc.dram_tensor("xT", [M, P], DT_MM, kind="ExternalInput").ap(),
        "wq": nc.dram_tensor("wq", [NPAIRS, MK, 128, 128], DT_MM,
                             kind="ExternalInput").ap(),
        "wk": nc.dram_tensor("wk", [NPAIRS, MK, 128, 128], DT_MM,
                             kind="ExternalInput").ap(),
        "wv": nc.dram_tensor("wv", [NPAIRS, MK, 128, 128], DT_MM,
                             kind="ExternalInput").ap(),
        "wo": nc.dram_tensor("wo", [NPAIRS, 128, 1024], DT_MM,
                             kind="ExternalInput").ap(),
        "bq": nc.dram_tensor("bq", [NPAIRS, 1, 128], DT_MM,
                             kind="ExternalInput").ap(),
        "bk": nc.dram_tensor("bk", [NPAIRS, 1, 128], DT_MM,
                             kind="ExternalInput").ap(),
        "bv": nc.dram_tensor("bv", [NPAIRS, 1, 128], DT_MM,
                             kind="ExternalInput").ap(),
        "eye": nc.dram_tensor("eye", [128, 128], F32,
                              kind="ExternalInput").ap(),
        "mask": nc.dram_tensor("mask", [128, 128], DT_MM,
                               kind="ExternalInput").ap(),
        "outp": nc.dram_tensor("outp", [P, M], F32, kind="ExternalOutput").ap(),
    }
    with tile.TileContext(nc) as tc:
        with ExitStack() as ctx:
            _emit(nc, tc, aps, ctx)
    nc.compile()
    _BUILT[DT_MODE] = nc
    return nc


def _host_inputs(x, kq, kk, kv, ko, bq, bk, bv):
    xT = np.ascontiguousarray(x.transpose(0, 2, 1)).astype(NP_MM)  # [B, M, P]
    eye = np.eye(128, dtype=np.float32)
    # keep iff pq < pk; block mask[r(pk), c(pq)] = 1 if c < r
    mask = np.tril(np.ones((128, 128), np.float32), k=-1).astype(NP_MM)
    in_maps = []
    for c in range(NCORES):
        b, k4 = divmod(c, 4)
        heads = [4 * k4 + i for i in range(HPC)]

        def pairw(kern):
            # [NPAIRS, MK, 128, 128] lhsT chunks
            out = np.empty((NPAIRS, MK, 128, 128), NP_MM)
            for pr in range(NPAIRS):
                pairm = np.concatenate(
                    [kern[heads[2 * pr]], kern[heads[2 * pr + 1]]], axis=1
                )  # [1024, 128]
                out[pr] = pairm.reshape(MK, 128, 128).astype(NP_MM)
            return out

        def pairb(bias):
            out = np.empty((NPAIRS, 1, 128), NP_MM)
            for pr in range(NPAIRS):
                out[pr, 0] = np.concatenate(
                    [bias[heads[2 * pr]], bias[heads[2 * pr + 1]]]
                ).astype(NP_MM)
            return out

        wo = np.empty((NPAIRS, 128, 1024), NP_MM)
        for pr in range(NPAIRS):
            wo[pr] = np.concatenate(
                [ko[heads[2 * pr]], ko[heads[2 * pr + 1]]], axis=0
            ).astype(NP_MM)

        in_maps.append({
            "xT": xT[b],
            "wq": pairw(kq), "wk": pairw(kk), "wv": pairw(kv),
            "wo": wo,
            "bq": pairb(bq), "bk": pairb(bk), "bv": pairb(bv),
            "eye": eye, "mask": mask,
        })
    return in_maps


def kernel(x, kernel_query, kernel_key, kernel_value, kernel_out,
           bias_query, bias_key, bias_value, bias_out, _trace=False):
    x = np.asarray(x, np.float32)
    kq = np.asarray(kernel_query, np.float32)
    kk = np.asarray(kernel_key, np.float32)
    kv = np.asarray(kernel_value, np.float32)
    ko = np.asarray(kernel_out, np.float32)
    bq = np.asarray(bias_query, np.float32)
    bk = np.asarray(bias_key, np.float32)
    bv = np.asarray(bias_value, np.float32)
    bo = np.asarray(bias_out, np.float32)

    nc = _build()
    in_maps = _host_inputs(x, kq, kk, kv, ko, bq, bk, bv)
    res = bass_utils.run_bass_kernel_spmd(
        nc, in_maps, core_ids=list(range(NCORES)), trace=_trace
    )
    out = np.zeros((B, P, M), np.float32)
    for c in range(NCORES):
        out[c // 4] += res.results[c]["outp"]
    out += bo[None, None, :]

    # patch fully-masked query row P-1: uniform attention = mean_k v
    for b in range(B):
        xbar = x[b].mean(axis=0, dtype=np.float64)  # [M]
        row = np.zeros(M, np.float64)
        for n in range(N):
            zrow = xbar @ kv[n].astype(np.float64) + bv[n].astype(np.float64)
            row += zrow @ ko[n].astype(np.float64)
        out[b, P - 1, :] = (row + bo.astype(np.float64)).astype(np.float32)

    if _trace:
        kernel._last_result = res
    return out
